# revision 28
# baseline (speedup 1.0000x reference)
"""Trainium2 Bass kernel v3: ragged phonology-embedding mean + position add.

Reference semantics (per (b, s)):
    out[b, s, :] = mean_{g < len[b,s]} table[tok[b,s,g], :] + pos[s, :]

Data-parallel over B across 8 cores; per core 16 output tiles of 128 rows.
The ragged mean is a matmul: W[p, m] = count/len, rhs = vocab rows.
Three vocab regions balance PE (matmul) time vs Q7 (gather desc-gen) time:

  - FIAT  (f chunks): vocab rows [0, 128f) read contiguously from the table
    (no gather); every tile matmuls them. Fills PE while the Q7 gather
    ucode library loads (~17us) and whenever gather data lags.
  - HOT   (h chunks): the 128h most popular remaining rows, gathered ONCE
    per core; every tile matmuls all h chunks.
  - COLD: per-tile leftovers, deduped per tile-PAIR with membership-ordered
    unions so most gathered chunks matmul into only 1-2 PSUM tiles.

v3 structural changes over v2 (91.9us -> target ~70us):
  - gather calls split to <= csz chunks: HW shows per-call overhead ~0,
    so fine calls stream data to PE in ~2.3us quanta (was 8.6us lumps,
    causing a 4-8us PE stall mid-run and a fat tail).
  - all input DMAs are DRAM-contiguous: fiat chunks come straight from
    table rows, cff/cfh are tile-major, W is ONE upfront 15KB/partition
    load (v2 sliced column loads decayed into 768B/partition descriptors,
    delaying first matmul to 6.1us).
  - hot gather can be deferred behind the first cold calls (hot matmuls
    only gate each tile's first entry, which lands later anyway).
  - psum tiles are [128,1024] f32 (2 banks); evictions are single DVE
    instructions; constants recalibrated from the v2 HW trace.
"""

import os
import numpy as np
import ml_dtypes

import concourse.bass as bass
import concourse.bacc as bacc
import concourse.mybir as mybir
import concourse.tile as tile
from concourse.bass_utils import run_bass_kernel_spmd

B, S, G = 128, 128, 8
VOCAB, D = 2048, 1024
NCORES = 8
BPC = B // NCORES
R = BPC * S
P = 128
NT = R // P                 # 16 output tiles per core
KT = VOCAB // P
MAXCH = 8                   # dma_gather HW cap: 1024 indices per call

# calibrated engine costs (ns), from v2/v3/v4 HW traces (exec-window frame)
LIB_END_NS = 15_500.0       # first gather can start (Q7 ucode lib ready)
Q7_NS_PER_IDX = 9.0         # joint desc-gen + SDMA drain rate per index
Q7_CALL_FIXED_NS = 340.0
DRAIN_NS = 2_200.0          # cumulative-idx position -> data usable by PE
PE_START_NS = 5_500.0       # first matmul (fiat chunk 0 + cff tile 0 DMA)
W_LAND_NS = 9_000.0         # one-shot W load complete (gates cold entries)
PE_NS_PER_CHUNK = 440.0     # [128x128] @ [128x1024] as 2 N=512 matmuls
FIAT_OVH_NS = 80.0
SLACK_NS = 1_500.0          # landing jitter absorbed before each call
DVE_PASS_NS = 1_300.0       # one [128,1024] evict pass
OUT_DMA_NS = 700.0
TEARDOWN_NS = 5_600.0       # final sem chain after last evict+store
GT_BUFS = 12


def _cdiv(a, b):
    return -(-a // b)


def _base_stats(phon_tokens, group_len_raw):
    toks = np.asarray(phon_tokens).astype(np.int64).reshape(B, S, G)
    lens = (np.asarray(group_len_raw).astype(np.int64) + 1).reshape(B, S)
    toks_c = toks.reshape(NCORES, R, G)
    lens_c = lens.reshape(NCORES, R)
    uniqs = {}
    wmats = {}   # integer counts (exact in fp8 e4m3); 1/len applied at evict
    kvs = np.zeros((NCORES, VOCAB), np.int64)
    for c in range(NCORES):
        for t in range(NT):
            tl = toks_c[c, t * P:(t + 1) * P]
            ll = lens_c[c, t * P:(t + 1) * P]
            valid = np.arange(G)[None, :] < ll[:, None]
            flat = tl[valid]
            pair = np.repeat(np.arange(P), ll)
            uniq, inv = np.unique(flat, return_inverse=True)
            wm = np.zeros((uniq.size, P), np.float32)
            np.add.at(wm, (inv, pair), 1.0)
            uniqs[c, t] = uniq
            wmats[c, t] = wm
            kvs[c, uniq] += 1
    leninv = (1.0 / lens_c).astype(np.float32)   # [NCORES, R]
    return uniqs, wmats, kvs, leninv


def _plan(uniqs, kvs, f, h, gsz, csz, hot_pos, hot_last=0):
    """Build regions / ordered group unions / split calls."""
    nfiat = 128 * f
    hot_rows = {}
    for c in range(NCORES):
        kv = kvs[c].copy()
        kv[:nfiat] = -1
        if h > 0:
            order = np.lexsort((np.arange(VOCAB), -kv))
            hot_rows[c] = np.sort(order[:128 * h])
        else:
            hot_rows[c] = np.zeros(0, np.int64)

    colds = {}
    for c in range(NCORES):
        hotset = hot_rows[c]
        for t in range(NT):
            u = uniqs[c, t]
            mask = u >= nfiat
            if h > 0:
                mask &= ~np.isin(u, hotset, assume_unique=True)
            colds[c, t] = u[mask]

    # one segment per tile-group with membership-ordered union
    ngroups = NT // gsz
    segments = []
    for gi in range(ngroups):
        tiles = list(range(gi * gsz, (gi + 1) * gsz))
        nU = np.zeros(NCORES, int)
        tok = {}
        msk = {}
        for c in range(NCORES):
            sets = [colds[c, t] for t in tiles]
            union = np.unique(np.concatenate(sets))
            m = np.zeros(union.size, np.int64)
            for bi, s in enumerate(sets):
                m[np.isin(union, s, assume_unique=True)] |= 1 << bi
            if gsz == 2:
                key = np.select([m == 1, m == 3, m == 2], [0, 1, 2])
            else:
                lowm = m & 3
                highm = (m >> 2) & 3
                blk = np.where(highm == 0, 0, np.where(lowm == 0, 2, 1))
                sub = np.select([m == 1, m == 3, m == 2], [0, 1, 2], 0)
                sub2 = np.select([m == 4, m == 12, m == 8], [0, 1, 2], 0)
                key = blk * 16 + np.where(blk == 0, sub, np.where(
                    blk == 2, sub2, m))
            order = np.lexsort((union, key))
            tok[c] = union[order]
            msk[c] = m[order]
            nU[c] = union.size
        nch = max(int(_cdiv(int(nU.max()), P)), 1)
        assert nch <= 2 * MAXCH, (nch, gi)
        segments.append(dict(tiles=tiles, tok=tok, msk=msk, nch=nch))

    # split every segment into calls of <= csz chunks
    calls = []
    tokpad = {}   # (core, call_idx) -> padded int64 [nch*P]
    chunk_off = h
    entry_off = 0
    for seg in segments:
        nch_seg = seg["nch"]
        for c0 in range(0, nch_seg, csz):
            c1 = min(c0 + csz, nch_seg)
            nch = c1 - c0
            gidx = len(calls)
            ent = set()
            for c in range(NCORES):
                tl = seg["tok"][c]
                m = seg["msk"][c]
                tp = np.zeros(nch * P, np.int64)
                lo, hi = c0 * P, min(c1 * P, tl.size)
                if hi > lo:
                    tp[:hi - lo] = tl[lo:hi]
                tokpad[c, gidx] = tp
                for j in range(nch):
                    slo, shi = (c0 + j) * P, min((c0 + j + 1) * P, m.size)
                    if shi <= slo:
                        continue
                    seg_m = m[slo:shi]
                    for bi, t in enumerate(seg["tiles"]):
                        if np.any(seg_m & (1 << bi)):
                            ent.add((j, t))
            entries = sorted(ent, key=lambda e: (e[0], e[1]))
            calls.append(dict(
                nch=nch, idx_base=chunk_off, entry_base=entry_off,
                entries=entries, grp=tuple(seg["tiles"]),
            ))
            chunk_off += nch
            entry_off += len(entries)

    hot_pos = min(hot_pos, len(calls))
    gfirst, glast = {}, {}
    for ci, call in enumerate(calls):
        for e, (j, t) in enumerate(call["entries"]):
            gfirst.setdefault(t, (ci, e))
            glast[t] = (ci, e)
    assert len(gfirst) == NT, "every tile needs >=1 cold entry"
    for ci, call in enumerate(calls):
        call["first"] = {t: e for t, (c_, e) in gfirst.items() if c_ == ci}
        call["last"] = {t: e for t, (c_, e) in glast.items() if c_ == ci}

    return dict(
        f=f, h=h, gsz=gsz, csz=csz, hot_pos=hot_pos, hot_last=hot_last,
        calls=calls, hot_rows=hot_rows, colds=colds, tokpad=tokpad,
        total_chunks=max(chunk_off, 1), total_entries=max(entry_off, 1),
    )


def _schedule(plan):
    """Greedy PE/Q7/DVE co-sim; returns (sched, makespan_est).

    sched is the PE emission order: ("fiat", t) and ("call", ci) items.
    Data landing is modeled as the joint desc-gen + SDMA pipe: call i's
    data is usable at LIB_END + (cumulative idx through i) * rate + DRAIN.
    """
    f, h = plan["f"], plan["h"]
    calls = plan["calls"]
    hot_pos = plan["hot_pos"]
    hot_last = plan["hot_last"]

    # Q7 pipe: calls[0:hot_pos], hot, calls[hot_pos:]
    q7 = LIB_END_NS
    land = [0.0] * len(calls)
    hot_land = 0.0
    for ci in range(hot_pos):
        q7 += calls[ci]["nch"] * P * Q7_NS_PER_IDX + Q7_CALL_FIXED_NS
        land[ci] = q7 + DRAIN_NS
    if h:
        q7 += 128 * h * Q7_NS_PER_IDX + Q7_CALL_FIXED_NS
        hot_land = q7 + DRAIN_NS
    for ci in range(hot_pos, len(calls)):
        q7 += calls[ci]["nch"] * P * Q7_NS_PER_IDX + Q7_CALL_FIXED_NS
        land[ci] = q7 + DRAIN_NS

    # deadline[t]: call index holding tile t's last entry.  The fiat burst
    # (and its DVE evict) MUST be emitted before that call's final evict,
    # or the in-order DVE queue deadlocks (final evict would wait on a
    # later-emitted fiat evict).
    deadline = {}
    for ci, call in enumerate(calls):
        for t in call["last"]:
            deadline[t] = ci

    pe = PE_START_NS          # simulated clock
    work = PE_START_NS        # cumulative emitted PE work (no-stall clock)
    dve = 0.0
    sched = []
    fiat_left = sorted(range(NT), key=lambda t: deadline.get(t, 0))
    end_of_tile = {}

    def emit_fiat(t):
        nonlocal pe, work, dve
        sched.append(("fiat", t))
        fiat_left.remove(t)
        pe = max(pe + f * PE_NS_PER_CHUNK + FIAT_OVH_NS, 0)
        work += f * PE_NS_PER_CHUNK + FIAT_OVH_NS
        dve = max(dve, pe) + DVE_PASS_NS

    for ci, call in enumerate(calls):
        # hold the call back behind cheap reserve work until its data is
        # predicted to have landed (plus slack) — an in-order PE stream
        # that arrives late never stalls.
        while fiat_left and work < land[ci] + SLACK_NS:
            emit_fiat(fiat_left[0])
        for t in call["grp"]:
            if t in fiat_left and deadline.get(t) == ci:
                emit_fiat(t)
        start = max(pe, land[ci], W_LAND_NS)
        for e, (j, t) in enumerate(call["entries"]):
            if call["first"].get(t, -1) == e and h and not hot_last:
                start = max(start, hot_land)
                start += h * PE_NS_PER_CHUNK
                work += h * PE_NS_PER_CHUNK
            start += PE_NS_PER_CHUNK
            work += PE_NS_PER_CHUNK
            if call["last"].get(t, -1) == e:
                if h and hot_last:
                    start = max(start, hot_land)
                    start += h * PE_NS_PER_CHUNK
                    work += h * PE_NS_PER_CHUNK
                end_of_tile[t] = start
        pe = start
        sched.append(("call", ci))
    while fiat_left:
        emit_fiat(fiat_left[0])
    # final evictions on the DVE queue, in tile-completion order
    for t in sorted(end_of_tile, key=end_of_tile.get):
        dve = max(dve, end_of_tile[t]) + DVE_PASS_NS
    mk = max(dve, pe) + OUT_DMA_NS + TEARDOWN_NS
    return sched, mk


def _materialize(plan, uniqs, wmats):
    """Build idx / w / cff / cfh numpy maps for each core.

    Weight matrices hold integer counts 0..8 — exact in fp8 e4m3, which
    halves their HBM/SBUF footprint (matmul fp8 lhsT x bf16 rhs runs at
    full bf16 speed and is exact for these values).
    """
    f, h = plan["f"], plan["h"]
    calls = plan["calls"]
    hot_rows = plan["hot_rows"]
    total_chunks = plan["total_chunks"]
    total_entries = plan["total_entries"]
    wdt = ml_dtypes.float8_e4m3

    idx_all = np.zeros((NCORES, total_chunks * P), np.int64)
    w_all = np.zeros((NCORES, total_entries, P, P), np.float32)
    cf_all = np.zeros((NCORES, NT, max(f + h, 1), P, P), np.float32)
    for c in range(NCORES):
        if h > 0:
            idx_all[c, :hot_rows[c].size] = hot_rows[c]
        for t in range(NT):
            u = uniqs[c, t]
            wm = wmats[c, t]
            for j in range(f):
                lo, hi = j * P, (j + 1) * P
                sel = (u >= lo) & (u < hi)
                if sel.any():
                    cf_all[c, t, j, u[sel] - lo] = wm[sel]
            hr = hot_rows[c]
            if h > 0:
                pos_in_u = np.minimum(np.searchsorted(u, hr), u.size - 1)
                ok = u[pos_in_u] == hr
                for j in range(h):
                    rows = np.arange(j * P, (j + 1) * P)
                    okj = ok[rows]
                    cf_all[c, t, f + j, np.nonzero(okj)[0]] = (
                        wm[pos_in_u[rows[okj]]]
                    )
        for gidx, call in enumerate(calls):
            toks_l = plan["tokpad"][c, gidx]
            b0 = call["idx_base"]
            idx_all[c, b0 * P:b0 * P + toks_l.size] = toks_l
            # padding slots hold token 0, which is always fiat (f >= 1), so
            # isin against the cold sets already excludes them
            in_t = {
                t: np.isin(toks_l, plan["colds"][c, t])
                for t in call["grp"]
            }
            uu = {t: uniqs[c, t] for t in call["grp"]}
            for e, (j, tt) in enumerate(call["entries"]):
                lo, hi = j * P, (j + 1) * P
                seg = toks_l[lo:hi]
                side = in_t[tt][lo:hi]
                if not side.any():
                    continue
                rows = np.searchsorted(uu[tt], seg[side])
                w_all[c, call["entry_base"] + e, np.nonzero(side)[0]] = (
                    wmats[c, tt][rows]
                )

    idx_maps, w_maps, cff_maps, cfh_maps = [], [], [], []
    for c in range(NCORES):
        idxw = np.tile(idx_all[c].reshape(-1, 16).T, (8, 1)).astype(np.int16)
        idx_maps.append(np.ascontiguousarray(idxw))
        # w: [P, TE*P] partition-major (one-shot load, 15KB/partition
        # contiguous, AP order matches the SBUF destination)
        wf = w_all[c].transpose(1, 0, 2).reshape(P, total_entries * P)
        w_maps.append(np.ascontiguousarray(wf.astype(wdt)))
        # cff/cfh: [NT*P, f*P] tile-major (contiguous per-tile slabs)
        cfp = cf_all[c][:, :max(f, 1)] if f else cf_all[c][:, :1]
        chp = cf_all[c][:, f:f + h] if h else cf_all[c][:, :1]
        cff_maps.append(np.ascontiguousarray(
            cfp.transpose(0, 2, 1, 3)
            .reshape(NT * P, max(f, 1) * P).astype(wdt)))
        cfh_maps.append(np.ascontiguousarray(
            chp.transpose(0, 2, 1, 3)
            .reshape(NT * P, max(h, 1) * P).astype(wdt)))
    return idx_maps, w_maps, cff_maps, cfh_maps


def _prepare(phon_tokens, group_len_raw):
    uniqs, wmats, kvs, leninv = _base_stats(phon_tokens, group_len_raw)
    fe = os.environ.get("F")
    if fe is not None:
        grid = [(int(fe), int(os.environ.get("H", 3)),
                 int(os.environ.get("GSZ", 2)),
                 int(os.environ.get("CSZ", 4)),
                 int(os.environ.get("HOTPOS", 0)),
                 int(os.environ.get("HOTLAST", 0)))]
    else:
        grid = [
            (ff, hh, 2, cc, 0, 0)
            for ff in (3, 4)
            for hh in (2, 3)
            for cc in (3, 4)
        ]
    best = None
    for (ff, hh, gg, cc, hp, hl) in grid:
        try:
            plan = _plan(uniqs, kvs, ff, hh, gg, cc, hp, hl)
        except AssertionError:
            continue
        sched, mk = _schedule(plan)
        if best is None or mk < best[0]:
            best = (mk, plan, sched)
    mk, plan, sched = best
    if os.environ.get("VERBOSE"):
        nidx = 128 * plan["h"] + sum(
            c["nch"] * P for c in plan["calls"])
        nent = sum(len(c["entries"]) for c in plan["calls"])
        print(f"[plan] f={plan['f']} h={plan['h']} gsz={plan['gsz']} "
              f"csz={plan['csz']} hotpos={plan['hot_pos']} "
              f"hotlast={plan['hot_last']} "
              f"makespan={mk/1000:.1f}us idx={nidx} entries={nent} "
              f"calls={len(plan['calls'])}")
    idx_maps, w_maps, cff_maps, cfh_maps = _materialize(plan, uniqs, wmats)
    leninv_maps = [
        np.ascontiguousarray(leninv[c].reshape(NT, P).T)
        for c in range(NCORES)
    ]
    meta = dict(plan=plan, sched=sched)
    return meta, idx_maps, w_maps, cff_maps, cfh_maps, leninv_maps


def _build_nc(meta):
    mdt = mybir.dt.bfloat16
    f32 = mybir.dt.float32
    plan = meta["plan"]
    sched = meta["sched"]
    f, h = plan["f"], plan["h"]
    calls = plan["calls"]
    hot_pos = plan["hot_pos"]
    total_chunks = plan["total_chunks"]
    total_entries = plan["total_entries"]
    csz = plan["csz"]

    f8 = mybir.dt.float8e4
    nc = bacc.Bacc("TRN2", target_bir_lowering=False, debug=False)

    table_d = nc.dram_tensor("table", [VOCAB, D], mdt, kind="ExternalInput")
    pos_d = nc.dram_tensor("pos", [P, D], f32, kind="ExternalInput")
    leninv_d = nc.dram_tensor("leninv", [P, NT], f32, kind="ExternalInput")
    idx_d = nc.dram_tensor("idxs", [P, total_chunks * 8], mybir.dt.int16,
                           kind="ExternalInput")
    w_d = nc.dram_tensor("wmat", [P, total_entries * P], f8,
                         kind="ExternalInput")
    cff_d = nc.dram_tensor("cff", [NT * P, max(f, 1) * P], f8,
                           kind="ExternalInput")
    cfh_d = nc.dram_tensor("cfh", [NT * P, max(h, 1) * P], f8,
                           kind="ExternalInput")
    out_d = nc.dram_tensor("out", [R, D], mdt, kind="ExternalOutput")

    fiat_order = [t for (k, t) in sched if k == "fiat"]
    gp_evict = set(calls[-1]["grp"]) if calls else set()

    with tile.TileContext(nc) as tc:
        with (
            tc.tile_pool(name="const", bufs=1) as cpool,
            tc.tile_pool(name="gather", bufs=GT_BUFS) as gpool,
            tc.tile_pool(name="osb", bufs=4) as opool,
            tc.tile_pool(name="psum", bufs=4, space=bass.MemorySpace.PSUM) as ppool,
        ):
            idx_sb = cpool.tile([P, total_chunks * 8], mybir.dt.int16)
            nregs = {}

            def _nreg(n):
                if n not in nregs:
                    nregs[n] = nc.gpsimd.to_reg(n)
                return nregs[n]

            # start-critical loads first: idx (gathers); fiat chunk 0 +
            # first cff tile slabs feed the first matmuls on sync; the W
            # one-shot goes early on scalar (it gates every cold entry).
            nc.scalar.dma_start(idx_sb[:], idx_d[:])
            cff_sb = None
            fiat_sb = None
            if f:
                fiat_sb = cpool.tile([P, f, D], mdt)
                nc.sync.dma_start(fiat_sb[:, 0, :], table_d[0:P, :])
                cff_sb = cpool.tile([P, NT, f * P], f8)
                for t in fiat_order[:2]:
                    nc.sync.dma_start(cff_sb[:, t, :],
                                      cff_d[t * P:(t + 1) * P, :])
                for j in range(1, f):
                    nc.sync.dma_start(fiat_sb[:, j, :],
                                      table_d[j * P:(j + 1) * P, :])
            pos_sb = cpool.tile([P, D], f32)
            nc.sync.dma_start(pos_sb[:], pos_d[:])
            leninv_sb = cpool.tile([P, NT], f32)
            nc.sync.dma_start(leninv_sb[:], leninv_d[:])
            if f:
                for t in fiat_order[2:]:
                    nc.sync.dma_start(cff_sb[:, t, :],
                                      cff_d[t * P:(t + 1) * P, :])
            wt_sb = cpool.tile([P, total_entries, P], f8)
            nc.scalar.dma_start(
                wt_sb[:, :, :],
                w_d[:, :],
            )
            cfh_sb = None
            if h:
                cfh_sb = cpool.tile([P, NT, h * P], f8)
                for t in range(NT):
                    nc.scalar.dma_start(cfh_sb[:, t, :],
                                        cfh_d[t * P:(t + 1) * P, :])

            hot_sb = cpool.tile([P, max(h, 1), D], mdt)

            def emit_hot_gather():
                nc.gpsimd.dma_gather(
                    hot_sb[:, :, :], table_d[:], idx_sb[:, :h * 8],
                    num_idxs=h * P, num_idxs_reg=_nreg(h * P), elem_size=D,
                )

            if h and hot_pos == 0:
                emit_hot_gather()

            out_sb = cpool.tile([P, NT, D], mdt)
            psums = {}
            ncalls_emitted = 0

            for kind, item in sched:
                if kind == "fiat":
                    t = item
                    ps = ppool.tile([P, 1024], f32, tag="ps", name="psf")
                    for j in range(f):
                        for hh in range(0, D, 512):
                            nc.tensor.matmul(
                                ps[:, hh:hh + 512],
                                lhsT=cff_sb[:, t, j * P:(j + 1) * P],
                                rhs=fiat_sb[:, j, hh:hh + 512],
                                start=(j == 0), stop=(j == f - 1),
                            )
                    nc.vector.scalar_tensor_tensor(
                        out_sb[:, t, :], ps[:, :], leninv_sb[:, t:t + 1],
                        pos_sb[:, :],
                        op0=mybir.AluOpType.mult, op1=mybir.AluOpType.add,
                    )
                else:
                    call = calls[item]
                    nch = call["nch"]
                    b0 = call["idx_base"]
                    n_idx = nch * P
                    gt = gpool.tile([P, csz, D], mdt, tag="gt")
                    nc.gpsimd.dma_gather(
                        gt[:, :nch, :], table_d[:],
                        idx_sb[:, b0 * 8:(b0 + nch) * 8],
                        num_idxs=n_idx, num_idxs_reg=_nreg(n_idx),
                        elem_size=D,
                    )
                    ncalls_emitted += 1
                    if h and hot_pos == ncalls_emitted:
                        emit_hot_gather()
                    eb = call["entry_base"]
                    hot_last = plan["hot_last"]

                    def emit_hot_mms(t, first):
                        for jj in range(h):
                            for hh in range(0, D, 512):
                                nc.tensor.matmul(
                                    psums[t][:, hh:hh + 512],
                                    lhsT=cfh_sb[:, t, jj * P:(jj + 1) * P],
                                    rhs=hot_sb[:, jj, hh:hh + 512],
                                    start=(first and jj == 0), stop=False,
                                )

                    for e, (j, t) in enumerate(call["entries"]):
                        is_first = call["first"].get(t, -1) == e
                        is_last = call["last"].get(t, -1) == e
                        if is_first:
                            psums[t] = ppool.tile([P, 1024], f32, tag="ps",
                                                  name="psc")
                            if h and not hot_last:
                                emit_hot_mms(t, True)
                        first_mm = bool(is_first and (h == 0 or hot_last))
                        if is_last and h and hot_last:
                            emit_hot_mms(t, first_mm)
                            first_mm = False
                        for hh in range(0, D, 512):
                            nc.tensor.matmul(
                                psums[t][:, hh:hh + 512],
                                lhsT=wt_sb[:, eb + e, :],
                                rhs=gt[:, j, hh:hh + 512],
                                start=first_mm,
                                stop=is_last,
                            )
                        if is_last:
                            ot = opool.tile([P, D], mdt, tag="ot")
                            ev_eng = nc.vector
                            ev_eng.scalar_tensor_tensor(
                                ot[:, :], psums[t][:, :],
                                leninv_sb[:, t:t + 1], out_sb[:, t, :],
                                op0=mybir.AluOpType.mult,
                                op1=mybir.AluOpType.add,
                            )
                            nc.sync.dma_start(
                                out_d[t * P:(t + 1) * P, :], ot[:]
                            )
    nc.compile()
    return nc


def run(inputs, trace=False, tmpdir=None):
    meta, idx_maps, w_maps, cff_maps, cfh_maps, leninv_maps = _prepare(
        inputs["phon_tokens"], inputs["group_len_raw"]
    )
    wdt = ml_dtypes.bfloat16
    table_np = np.ascontiguousarray(
        np.asarray(inputs["phon_emb_table"]).astype(wdt)
    )
    pos_np = np.ascontiguousarray(
        np.asarray(inputs["pos_emb_table"]).astype(np.float32)
    )

    nc = _build_nc(meta)
    in_maps = [
        {
            "table": table_np, "pos": pos_np, "leninv": leninv_maps[c],
            "idxs": idx_maps[c], "wmat": w_maps[c],
            "cff": cff_maps[c], "cfh": cfh_maps[c],
        }
        for c in range(NCORES)
    ]
    res = run_bass_kernel_spmd(
        nc, in_maps, core_ids=list(range(NCORES)), trace=trace, tmpdir=tmpdir
    )
    out = np.empty((B, S, D), np.float32)
    for c in range(NCORES):
        out[c * BPC:(c + 1) * BPC] = (
            res.results[c]["out"].astype(np.float32).reshape(BPC, S, D)
        )
    return out, res


def kernel(**inputs) -> np.ndarray:
    out, _ = run(inputs, trace=False)
    return out


# revision 29
# speedup vs baseline: 1.0303x; 1.0303x over previous
"""Trainium2 Bass kernel v3: ragged phonology-embedding mean + position add.

Reference semantics (per (b, s)):
    out[b, s, :] = mean_{g < len[b,s]} table[tok[b,s,g], :] + pos[s, :]

Data-parallel over B across 8 cores; per core 16 output tiles of 128 rows.
The ragged mean is a matmul: W[p, m] = count/len, rhs = vocab rows.
Three vocab regions balance PE (matmul) time vs Q7 (gather desc-gen) time:

  - FIAT  (f chunks): vocab rows [0, 128f) read contiguously from the table
    (no gather); every tile matmuls them. Fills PE while the Q7 gather
    ucode library loads (~17us) and whenever gather data lags.
  - HOT   (h chunks): the 128h most popular remaining rows, gathered ONCE
    per core; every tile matmuls all h chunks.
  - COLD: per-tile leftovers, deduped per tile-PAIR with membership-ordered
    unions so most gathered chunks matmul into only 1-2 PSUM tiles.

v3 structural changes over v2 (91.9us -> target ~70us):
  - gather calls split to <= csz chunks: HW shows per-call overhead ~0,
    so fine calls stream data to PE in ~2.3us quanta (was 8.6us lumps,
    causing a 4-8us PE stall mid-run and a fat tail).
  - all input DMAs are DRAM-contiguous: fiat chunks come straight from
    table rows, cff/cfh are tile-major, W is ONE upfront 15KB/partition
    load (v2 sliced column loads decayed into 768B/partition descriptors,
    delaying first matmul to 6.1us).
  - hot gather can be deferred behind the first cold calls (hot matmuls
    only gate each tile's first entry, which lands later anyway).
  - psum tiles are [128,1024] f32 (2 banks); evictions are single DVE
    instructions; constants recalibrated from the v2 HW trace.
"""

import os
import numpy as np
import ml_dtypes

import concourse.bass as bass
import concourse.bacc as bacc
import concourse.mybir as mybir
import concourse.tile as tile
from concourse.bass_utils import run_bass_kernel_spmd

B, S, G = 128, 128, 8
VOCAB, D = 2048, 1024
NCORES = 8
BPC = B // NCORES
R = BPC * S
P = 128
NT = R // P                 # 16 output tiles per core
KT = VOCAB // P
MAXCH = 8                   # dma_gather HW cap: 1024 indices per call

# calibrated engine costs (ns), from v2/v3/v4 HW traces (exec-window frame)
LIB_END_NS = 15_500.0       # first gather can start (Q7 ucode lib ready)
Q7_NS_PER_IDX = 9.0         # joint desc-gen + SDMA drain rate per index
Q7_CALL_FIXED_NS = 340.0
DRAIN_NS = 2_200.0          # cumulative-idx position -> data usable by PE
PE_START_NS = 5_500.0       # first matmul (fiat chunk 0 + cff tile 0 DMA)
W_LAND_NS = 9_000.0         # one-shot W load complete (gates cold entries)
PE_NS_PER_CHUNK = 440.0     # [128x128] @ [128x1024] as 2 N=512 matmuls
FIAT_OVH_NS = 80.0
SLACK_NS = 1_500.0          # landing jitter absorbed before each call
DVE_PASS_NS = 1_300.0       # one [128,1024] evict pass
OUT_DMA_NS = 700.0
TEARDOWN_NS = 5_600.0       # final sem chain after last evict+store
GT_BUFS = 12


def _cdiv(a, b):
    return -(-a // b)


def _base_stats(phon_tokens, group_len_raw):
    toks = np.asarray(phon_tokens).astype(np.int64).reshape(B, S, G)
    lens = (np.asarray(group_len_raw).astype(np.int64) + 1).reshape(B, S)
    toks_c = toks.reshape(NCORES, R, G)
    lens_c = lens.reshape(NCORES, R)
    uniqs = {}
    wmats = {}   # integer counts (exact in fp8 e4m3); 1/len applied at evict
    kvs = np.zeros((NCORES, VOCAB), np.int64)
    for c in range(NCORES):
        for t in range(NT):
            tl = toks_c[c, t * P:(t + 1) * P]
            ll = lens_c[c, t * P:(t + 1) * P]
            valid = np.arange(G)[None, :] < ll[:, None]
            flat = tl[valid]
            pair = np.repeat(np.arange(P), ll)
            uniq, inv = np.unique(flat, return_inverse=True)
            wm = np.zeros((uniq.size, P), np.float32)
            np.add.at(wm, (inv, pair), 1.0)
            uniqs[c, t] = uniq
            wmats[c, t] = wm
            kvs[c, uniq] += 1
    leninv = (1.0 / lens_c).astype(np.float32)   # [NCORES, R]
    return uniqs, wmats, kvs, leninv


def _plan(uniqs, kvs, f, h, gsz, csz, hot_pos, hot_last=0):
    """Build regions / ordered group unions / split calls."""
    nfiat = 128 * f
    hot_rows = {}
    for c in range(NCORES):
        kv = kvs[c].copy()
        kv[:nfiat] = -1
        if h > 0:
            order = np.lexsort((np.arange(VOCAB), -kv))
            hot_rows[c] = np.sort(order[:128 * h])
        else:
            hot_rows[c] = np.zeros(0, np.int64)

    colds = {}
    for c in range(NCORES):
        hotset = hot_rows[c]
        for t in range(NT):
            u = uniqs[c, t]
            mask = u >= nfiat
            if h > 0:
                mask &= ~np.isin(u, hotset, assume_unique=True)
            colds[c, t] = u[mask]

    # one segment per tile-group with membership-ordered union
    ngroups = NT // gsz
    segments = []
    for gi in range(ngroups):
        tiles = list(range(gi * gsz, (gi + 1) * gsz))
        nU = np.zeros(NCORES, int)
        tok = {}
        msk = {}
        for c in range(NCORES):
            sets = [colds[c, t] for t in tiles]
            union = np.unique(np.concatenate(sets))
            m = np.zeros(union.size, np.int64)
            for bi, s in enumerate(sets):
                m[np.isin(union, s, assume_unique=True)] |= 1 << bi
            if gsz == 2:
                key = np.select([m == 1, m == 3, m == 2], [0, 1, 2])
            else:
                lowm = m & 3
                highm = (m >> 2) & 3
                blk = np.where(highm == 0, 0, np.where(lowm == 0, 2, 1))
                sub = np.select([m == 1, m == 3, m == 2], [0, 1, 2], 0)
                sub2 = np.select([m == 4, m == 12, m == 8], [0, 1, 2], 0)
                key = blk * 16 + np.where(blk == 0, sub, np.where(
                    blk == 2, sub2, m))
            order = np.lexsort((union, key))
            tok[c] = union[order]
            msk[c] = m[order]
            nU[c] = union.size
        nch = max(int(_cdiv(int(nU.max()), P)), 1)
        assert nch <= 2 * MAXCH, (nch, gi)
        segments.append(dict(tiles=tiles, tok=tok, msk=msk, nch=nch))

    # split every segment into calls of <= csz chunks
    calls = []
    tokpad = {}   # (core, call_idx) -> padded int64 [nch*P]
    chunk_off = h
    entry_off = 0
    for seg in segments:
        nch_seg = seg["nch"]
        for c0 in range(0, nch_seg, csz):
            c1 = min(c0 + csz, nch_seg)
            nch = c1 - c0
            gidx = len(calls)
            ent = set()
            for c in range(NCORES):
                tl = seg["tok"][c]
                m = seg["msk"][c]
                tp = np.zeros(nch * P, np.int64)
                lo, hi = c0 * P, min(c1 * P, tl.size)
                if hi > lo:
                    tp[:hi - lo] = tl[lo:hi]
                tokpad[c, gidx] = tp
                for j in range(nch):
                    slo, shi = (c0 + j) * P, min((c0 + j + 1) * P, m.size)
                    if shi <= slo:
                        continue
                    seg_m = m[slo:shi]
                    for bi, t in enumerate(seg["tiles"]):
                        if np.any(seg_m & (1 << bi)):
                            ent.add((j, t))
            entries = sorted(ent, key=lambda e: (e[0], e[1]))
            calls.append(dict(
                nch=nch, idx_base=chunk_off, entry_base=entry_off,
                entries=entries, grp=tuple(seg["tiles"]),
            ))
            chunk_off += nch
            entry_off += len(entries)

    hot_pos = min(hot_pos, len(calls))
    gfirst, glast = {}, {}
    for ci, call in enumerate(calls):
        for e, (j, t) in enumerate(call["entries"]):
            gfirst.setdefault(t, (ci, e))
            glast[t] = (ci, e)
    assert len(gfirst) == NT, "every tile needs >=1 cold entry"
    for ci, call in enumerate(calls):
        call["first"] = {t: e for t, (c_, e) in gfirst.items() if c_ == ci}
        call["last"] = {t: e for t, (c_, e) in glast.items() if c_ == ci}

    return dict(
        f=f, h=h, gsz=gsz, csz=csz, hot_pos=hot_pos, hot_last=hot_last,
        calls=calls, hot_rows=hot_rows, colds=colds, tokpad=tokpad,
        total_chunks=max(chunk_off, 1), total_entries=max(entry_off, 1),
    )


def _schedule(plan):
    """Greedy PE/Q7/DVE co-sim; returns (sched, makespan_est).

    sched is the PE emission order: ("fiat", t) and ("call", ci) items.
    Data landing is modeled as the joint desc-gen + SDMA pipe: call i's
    data is usable at LIB_END + (cumulative idx through i) * rate + DRAIN.
    """
    f, h = plan["f"], plan["h"]
    calls = plan["calls"]
    hot_pos = plan["hot_pos"]
    hot_last = plan["hot_last"]

    # Q7 pipe: calls[0:hot_pos], hot, calls[hot_pos:]
    q7 = LIB_END_NS
    land = [0.0] * len(calls)
    hot_land = 0.0
    for ci in range(hot_pos):
        q7 += calls[ci]["nch"] * P * Q7_NS_PER_IDX + Q7_CALL_FIXED_NS
        land[ci] = q7 + DRAIN_NS
    if h:
        q7 += 128 * h * Q7_NS_PER_IDX + Q7_CALL_FIXED_NS
        hot_land = q7 + DRAIN_NS
    for ci in range(hot_pos, len(calls)):
        q7 += calls[ci]["nch"] * P * Q7_NS_PER_IDX + Q7_CALL_FIXED_NS
        land[ci] = q7 + DRAIN_NS

    # deadline[t]: call index holding tile t's last entry.  The fiat burst
    # (and its DVE evict) MUST be emitted before that call's final evict,
    # or the in-order DVE queue deadlocks (final evict would wait on a
    # later-emitted fiat evict).
    deadline = {}
    for ci, call in enumerate(calls):
        for t in call["last"]:
            deadline[t] = ci

    pe = PE_START_NS          # simulated clock
    work = PE_START_NS        # cumulative emitted PE work (no-stall clock)
    dve = 0.0
    sched = []
    fiat_left = sorted(range(NT), key=lambda t: deadline.get(t, 0))
    end_of_tile = {}

    def emit_fiat(t):
        nonlocal pe, work, dve
        sched.append(("fiat", t))
        fiat_left.remove(t)
        pe = max(pe + f * PE_NS_PER_CHUNK + FIAT_OVH_NS, 0)
        work += f * PE_NS_PER_CHUNK + FIAT_OVH_NS
        dve = max(dve, pe) + DVE_PASS_NS

    for ci, call in enumerate(calls):
        # hold the call back behind cheap reserve work until its data is
        # predicted to have landed (plus slack) — an in-order PE stream
        # that arrives late never stalls.
        while fiat_left and work < land[ci] + SLACK_NS:
            emit_fiat(fiat_left[0])
        for t in call["grp"]:
            if t in fiat_left and deadline.get(t) == ci:
                emit_fiat(t)
        start = max(pe, land[ci], W_LAND_NS)
        for e, (j, t) in enumerate(call["entries"]):
            if call["first"].get(t, -1) == e and h and not hot_last:
                start = max(start, hot_land)
                start += h * PE_NS_PER_CHUNK
                work += h * PE_NS_PER_CHUNK
            start += PE_NS_PER_CHUNK
            work += PE_NS_PER_CHUNK
            if call["last"].get(t, -1) == e:
                if h and hot_last:
                    start = max(start, hot_land)
                    start += h * PE_NS_PER_CHUNK
                    work += h * PE_NS_PER_CHUNK
                end_of_tile[t] = start
        pe = start
        sched.append(("call", ci))
    while fiat_left:
        emit_fiat(fiat_left[0])
    # final evictions on the DVE queue, in tile-completion order
    for t in sorted(end_of_tile, key=end_of_tile.get):
        dve = max(dve, end_of_tile[t]) + DVE_PASS_NS
    mk = max(dve, pe) + OUT_DMA_NS + TEARDOWN_NS
    return sched, mk


def _materialize(plan, uniqs, wmats):
    """Build idx / w / cff / cfh numpy maps for each core.

    Weight matrices hold integer counts 0..8 — exact in fp8 e4m3, which
    halves their HBM/SBUF footprint (matmul fp8 lhsT x bf16 rhs runs at
    full bf16 speed and is exact for these values).
    """
    f, h = plan["f"], plan["h"]
    calls = plan["calls"]
    hot_rows = plan["hot_rows"]
    total_chunks = plan["total_chunks"]
    total_entries = plan["total_entries"]
    wdt = ml_dtypes.float8_e4m3

    idx_all = np.zeros((NCORES, total_chunks * P), np.int64)
    w_all = np.zeros((NCORES, total_entries, P, P), np.float32)
    cf_all = np.zeros((NCORES, NT, max(f + h, 1), P, P), np.float32)
    for c in range(NCORES):
        if h > 0:
            idx_all[c, :hot_rows[c].size] = hot_rows[c]
        for t in range(NT):
            u = uniqs[c, t]
            wm = wmats[c, t]
            for j in range(f):
                lo, hi = j * P, (j + 1) * P
                sel = (u >= lo) & (u < hi)
                if sel.any():
                    cf_all[c, t, j, u[sel] - lo] = wm[sel]
            hr = hot_rows[c]
            if h > 0:
                pos_in_u = np.minimum(np.searchsorted(u, hr), u.size - 1)
                ok = u[pos_in_u] == hr
                for j in range(h):
                    rows = np.arange(j * P, (j + 1) * P)
                    okj = ok[rows]
                    cf_all[c, t, f + j, np.nonzero(okj)[0]] = (
                        wm[pos_in_u[rows[okj]]]
                    )
        for gidx, call in enumerate(calls):
            toks_l = plan["tokpad"][c, gidx]
            b0 = call["idx_base"]
            idx_all[c, b0 * P:b0 * P + toks_l.size] = toks_l
            # padding slots hold token 0, which is always fiat (f >= 1), so
            # isin against the cold sets already excludes them
            in_t = {
                t: np.isin(toks_l, plan["colds"][c, t])
                for t in call["grp"]
            }
            uu = {t: uniqs[c, t] for t in call["grp"]}
            for e, (j, tt) in enumerate(call["entries"]):
                lo, hi = j * P, (j + 1) * P
                seg = toks_l[lo:hi]
                side = in_t[tt][lo:hi]
                if not side.any():
                    continue
                rows = np.searchsorted(uu[tt], seg[side])
                w_all[c, call["entry_base"] + e, np.nonzero(side)[0]] = (
                    wmats[c, tt][rows]
                )

    idx_maps, w_maps, cff_maps, cfh_maps = [], [], [], []
    for c in range(NCORES):
        idxw = np.tile(idx_all[c].reshape(-1, 16).T, (8, 1)).astype(np.int16)
        idx_maps.append(np.ascontiguousarray(idxw))
        # w: [P, TE*P] partition-major (one-shot load, 15KB/partition
        # contiguous, AP order matches the SBUF destination)
        wf = w_all[c].transpose(1, 0, 2).reshape(P, total_entries * P)
        w_maps.append(np.ascontiguousarray(wf.astype(wdt)))
        # cff/cfh: [NT*P, f*P] tile-major (contiguous per-tile slabs)
        cfp = cf_all[c][:, :max(f, 1)] if f else cf_all[c][:, :1]
        chp = cf_all[c][:, f:f + h] if h else cf_all[c][:, :1]
        cff_maps.append(np.ascontiguousarray(
            cfp.transpose(0, 2, 1, 3)
            .reshape(NT * P, max(f, 1) * P).astype(wdt)))
        cfh_maps.append(np.ascontiguousarray(
            chp.transpose(0, 2, 1, 3)
            .reshape(NT * P, max(h, 1) * P).astype(wdt)))
    return idx_maps, w_maps, cff_maps, cfh_maps


def _prepare(phon_tokens, group_len_raw):
    uniqs, wmats, kvs, leninv = _base_stats(phon_tokens, group_len_raw)
    fe = os.environ.get("F")
    if fe is not None:
        grid = [(int(fe), int(os.environ.get("H", 3)),
                 int(os.environ.get("GSZ", 2)),
                 int(os.environ.get("CSZ", 4)),
                 int(os.environ.get("HOTPOS", 0)),
                 int(os.environ.get("HOTLAST", 0)))]
    else:
        grid = [
            (ff, hh, 2, cc, 0, 0)
            for ff in (3, 4)
            for hh in (2, 3)
            for cc in (3, 4)
        ]
    best = None
    for (ff, hh, gg, cc, hp, hl) in grid:
        try:
            plan = _plan(uniqs, kvs, ff, hh, gg, cc, hp, hl)
        except AssertionError:
            continue
        sched, mk = _schedule(plan)
        if best is None or mk < best[0]:
            best = (mk, plan, sched)
    mk, plan, sched = best
    if os.environ.get("VERBOSE"):
        nidx = 128 * plan["h"] + sum(
            c["nch"] * P for c in plan["calls"])
        nent = sum(len(c["entries"]) for c in plan["calls"])
        print(f"[plan] f={plan['f']} h={plan['h']} gsz={plan['gsz']} "
              f"csz={plan['csz']} hotpos={plan['hot_pos']} "
              f"hotlast={plan['hot_last']} "
              f"makespan={mk/1000:.1f}us idx={nidx} entries={nent} "
              f"calls={len(plan['calls'])}")
    idx_maps, w_maps, cff_maps, cfh_maps = _materialize(plan, uniqs, wmats)
    leninv_maps = [
        np.ascontiguousarray(leninv[c].reshape(NT, P).T)
        for c in range(NCORES)
    ]
    meta = dict(plan=plan, sched=sched)
    return meta, idx_maps, w_maps, cff_maps, cfh_maps, leninv_maps


def _build_nc(meta):
    mdt = mybir.dt.bfloat16
    f32 = mybir.dt.float32
    plan = meta["plan"]
    sched = meta["sched"]
    f, h = plan["f"], plan["h"]
    calls = plan["calls"]
    hot_pos = plan["hot_pos"]
    total_chunks = plan["total_chunks"]
    total_entries = plan["total_entries"]
    csz = plan["csz"]

    f8 = mybir.dt.float8e4
    nc = bacc.Bacc("TRN2", target_bir_lowering=False, debug=False)

    table_d = nc.dram_tensor("table", [VOCAB, D], mdt, kind="ExternalInput")
    pos_d = nc.dram_tensor("pos", [P, D], f32, kind="ExternalInput")
    leninv_d = nc.dram_tensor("leninv", [P, NT], f32, kind="ExternalInput")
    idx_d = nc.dram_tensor("idxs", [P, total_chunks * 8], mybir.dt.int16,
                           kind="ExternalInput")
    w_d = nc.dram_tensor("wmat", [P, total_entries * P], f8,
                         kind="ExternalInput")
    cff_d = nc.dram_tensor("cff", [NT * P, max(f, 1) * P], f8,
                           kind="ExternalInput")
    cfh_d = nc.dram_tensor("cfh", [NT * P, max(h, 1) * P], f8,
                           kind="ExternalInput")
    out_d = nc.dram_tensor("out", [R, D], mdt, kind="ExternalOutput")

    fiat_order = [t for (k, t) in sched if k == "fiat"]
    gp_evict = set(calls[-1]["grp"]) if calls else set()

    with tile.TileContext(nc) as tc:
        with (
            tc.tile_pool(name="const", bufs=1) as cpool,
            tc.tile_pool(name="gather", bufs=GT_BUFS) as gpool,
            tc.tile_pool(name="osb", bufs=4) as opool,
            tc.tile_pool(name="psum", bufs=4, space=bass.MemorySpace.PSUM) as ppool,
        ):
            idx_sb = cpool.tile([P, total_chunks * 8], mybir.dt.int16)
            nregs = {}

            def _nreg(n):
                if n not in nregs:
                    nregs[n] = nc.gpsimd.to_reg(n)
                return nregs[n]

            # start-critical loads first: idx (gathers); fiat chunk 0 +
            # first cff tile slabs feed the first matmuls on sync; the W
            # one-shot goes early on scalar (it gates every cold entry).
            nc.scalar.dma_start(idx_sb[:], idx_d[:])
            cff_sb = None
            fiat_sb = None
            if f:
                cff_sb = cpool.tile([P, NT, f * P], f8)
                ft = fiat_order[0]
                nc.sync.dma_start(cff_sb[:, ft, :],
                                  cff_d[ft * P:(ft + 1) * P, :])
                fiat_sb = cpool.tile([P, f, D], mdt)
                nc.sync.dma_start(fiat_sb[:, 0, :512], table_d[0:P, :512])
                nc.sync.dma_start(fiat_sb[:, 0, 512:], table_d[0:P, 512:])
                for t in fiat_order[1:2]:
                    nc.sync.dma_start(cff_sb[:, t, :],
                                      cff_d[t * P:(t + 1) * P, :])
                for j in range(1, f):
                    nc.sync.dma_start(fiat_sb[:, j, :],
                                      table_d[j * P:(j + 1) * P, :])
            pos_sb = cpool.tile([P, D], f32)
            nc.sync.dma_start(pos_sb[:], pos_d[:])
            leninv_sb = cpool.tile([P, NT], f32)
            nc.sync.dma_start(leninv_sb[:], leninv_d[:])
            if f:
                for t in fiat_order[2:]:
                    nc.sync.dma_start(cff_sb[:, t, :],
                                      cff_d[t * P:(t + 1) * P, :])
            wt_sb = cpool.tile([P, total_entries, P], f8)
            nc.scalar.dma_start(
                wt_sb[:, :, :],
                w_d[:, :],
            )
            cfh_sb = None
            if h:
                cfh_sb = cpool.tile([P, NT, h * P], f8)
                for t in range(NT):
                    nc.scalar.dma_start(cfh_sb[:, t, :],
                                        cfh_d[t * P:(t + 1) * P, :])

            hot_sb = cpool.tile([P, max(h, 1), D], mdt)

            def emit_hot_gather():
                nc.gpsimd.dma_gather(
                    hot_sb[:, :, :], table_d[:], idx_sb[:, :h * 8],
                    num_idxs=h * P, num_idxs_reg=_nreg(h * P), elem_size=D,
                )

            if h and hot_pos == 0:
                emit_hot_gather()

            out_sb = cpool.tile([P, NT, D], mdt)
            psums = {}
            ncalls_emitted = 0

            for kind, item in sched:
                if kind == "fiat":
                    t = item
                    ps = ppool.tile([P, 1024], f32, tag="ps", name="psf")
                    for j in range(f):
                        for hh in range(0, D, 512):
                            nc.tensor.matmul(
                                ps[:, hh:hh + 512],
                                lhsT=cff_sb[:, t, j * P:(j + 1) * P],
                                rhs=fiat_sb[:, j, hh:hh + 512],
                                start=(j == 0), stop=(j == f - 1),
                            )
                    for hh in range(0, D, 512):
                        nc.vector.scalar_tensor_tensor(
                            out_sb[:, t, hh:hh + 512], ps[:, hh:hh + 512],
                            leninv_sb[:, t:t + 1], pos_sb[:, hh:hh + 512],
                            op0=mybir.AluOpType.mult,
                            op1=mybir.AluOpType.add,
                        )
                else:
                    call = calls[item]
                    nch = call["nch"]
                    b0 = call["idx_base"]
                    n_idx = nch * P
                    gt = gpool.tile([P, csz, D], mdt, tag="gt")
                    nc.gpsimd.dma_gather(
                        gt[:, :nch, :], table_d[:],
                        idx_sb[:, b0 * 8:(b0 + nch) * 8],
                        num_idxs=n_idx, num_idxs_reg=_nreg(n_idx),
                        elem_size=D,
                    )
                    ncalls_emitted += 1
                    if h and hot_pos == ncalls_emitted:
                        emit_hot_gather()
                    eb = call["entry_base"]
                    hot_last = plan["hot_last"]

                    def emit_hot_mms(t, first):
                        for jj in range(h):
                            for hh in range(0, D, 512):
                                nc.tensor.matmul(
                                    psums[t][:, hh:hh + 512],
                                    lhsT=cfh_sb[:, t, jj * P:(jj + 1) * P],
                                    rhs=hot_sb[:, jj, hh:hh + 512],
                                    start=(first and jj == 0), stop=False,
                                )

                    for e, (j, t) in enumerate(call["entries"]):
                        is_first = call["first"].get(t, -1) == e
                        is_last = call["last"].get(t, -1) == e
                        if is_first:
                            psums[t] = ppool.tile([P, 1024], f32, tag="ps",
                                                  name="psc")
                            if h and not hot_last:
                                emit_hot_mms(t, True)
                        first_mm = bool(is_first and (h == 0 or hot_last))
                        if is_last and h and hot_last:
                            emit_hot_mms(t, first_mm)
                            first_mm = False
                        for hh in range(0, D, 512):
                            nc.tensor.matmul(
                                psums[t][:, hh:hh + 512],
                                lhsT=wt_sb[:, eb + e, :],
                                rhs=gt[:, j, hh:hh + 512],
                                start=first_mm,
                                stop=is_last,
                            )
                        if is_last:
                            ot = opool.tile([P, D], mdt, tag="ot")
                            for hh in range(0, D, 512):
                                nc.vector.scalar_tensor_tensor(
                                    ot[:, hh:hh + 512],
                                    psums[t][:, hh:hh + 512],
                                    leninv_sb[:, t:t + 1],
                                    out_sb[:, t, hh:hh + 512],
                                    op0=mybir.AluOpType.mult,
                                    op1=mybir.AluOpType.add,
                                )
                            nc.sync.dma_start(
                                out_d[t * P:(t + 1) * P, :], ot[:]
                            )
    nc.compile()
    return nc


def run(inputs, trace=False, tmpdir=None):
    meta, idx_maps, w_maps, cff_maps, cfh_maps, leninv_maps = _prepare(
        inputs["phon_tokens"], inputs["group_len_raw"]
    )
    wdt = ml_dtypes.bfloat16
    table_np = np.ascontiguousarray(
        np.asarray(inputs["phon_emb_table"]).astype(wdt)
    )
    pos_np = np.ascontiguousarray(
        np.asarray(inputs["pos_emb_table"]).astype(np.float32)
    )

    nc = _build_nc(meta)
    in_maps = [
        {
            "table": table_np, "pos": pos_np, "leninv": leninv_maps[c],
            "idxs": idx_maps[c], "wmat": w_maps[c],
            "cff": cff_maps[c], "cfh": cfh_maps[c],
        }
        for c in range(NCORES)
    ]
    res = run_bass_kernel_spmd(
        nc, in_maps, core_ids=list(range(NCORES)), trace=trace, tmpdir=tmpdir
    )
    out = np.empty((B, S, D), np.float32)
    for c in range(NCORES):
        out[c * BPC:(c + 1) * BPC] = (
            res.results[c]["out"].astype(np.float32).reshape(BPC, S, D)
        )
    return out, res


def kernel(**inputs) -> np.ndarray:
    out, _ = run(inputs, trace=False)
    return out
